# revision 27
# baseline (speedup 1.0000x reference)
"""Trainium2 Bass kernel for nn_BiRNNImputerModel (bidirectional GRU imputer).

Strategy (v2 — fold-batched gate math):
  - 8 cores: cores 0-3 run the forward GRU, cores 4-7 the backward GRU
    (backward = same program on time-reversed inputs).
  - Within a direction, data-parallel over batch: 128 / 4 = 32 per core.
  - On-chip layout is "transposed" [feature/H, batch] so recurrent matmuls
    need no per-step transposes; the 4 H-folds (512 = 4*128) of each gate
    live in the FREE dim of one PSUM bank: bank = [128, 4 folds * 32 batch].
    Gate nonlinearities then run as ONE [128,128] ACTIVATE per gate instead
    of 4 x [128,32] — the scalar/vector fixed per-instruction overhead
    (~300ns) dominated the old per-fold version.
  - Per-fold gate biases can't ride the ACTIVATE bias port (bias varies
    along the free dim), so each gate bank is seeded by a tiny K=4
    "indicator" matmul (stationary = 4 stacked fold-biases, moving = 0/1
    fold indicator) as the bank's start=True first write.
  - Input x/mask are SBUF-resident for the whole sequence, stored as
    [x ; 1-m] with the mask-half of Wih negated and sum_f Wih_m[:,f] folded
    into the biases. The per-step imputation x_p = m ? x : xhat is then a
    single copy_predicated that overwrites x in place (pred = 1-m) with
    xhat; the resident column block IS the gi matmul moving operand.
  - The per-step readout matmul uses a stacked stationary [Wro.T | WoutX.T]
    producing xhat_t and this direction's partial of the final
    bidirectional readout in one accumulation. bro rides the output-copy
    ACTIVATE's per-partition bias port.
  - Cross-direction sum + bout + layout fixes happen on the host; no
    cross-core communication.

PSUM discipline: every PSUM tile is padded to a full 2KB bank. Each bank
gets exactly ONE start=True per step (its seeding bias matmul / first
readout matmul); all other matmuls use start=False and rely on per-element
has_written accumulate-or-overwrite semantics.
"""

import os
import sys

for _p in ("/opt/trn_rl_repo", "/root/.axon_site/_ro/trn_rl_repo"):
    if os.path.isdir(_p) and _p not in sys.path:
        sys.path.insert(0, _p)

import numpy as np
import ml_dtypes

import concourse.bass as bass
import concourse.tile as tile
from concourse import mybir
from concourse.bass_utils import run_bass_kernel_spmd

BF16 = ml_dtypes.bfloat16

B, S, N, C = 128, 512, 64, 1
F = N * C          # 64
H = 512
NB = 32            # batch per core (128 / 4)
NFOLD = 4          # H / 128
CH = 64            # steps per resident-input chunk
WARM = 64          # chain-B warm-start steps (state error decays ~0.8^WARM)
AF = mybir.ActivationFunctionType
ALU = mybir.AluOpType


def _legalize_multiwait(nc, max_waits=1):
    """walrus in this image only encodes one sync-wait per instruction;
    hoist extra waits onto preceding NoOps."""
    n_fix = 0
    for f in nc.m.functions:
        for blk in f.blocks:
            new = []
            for ins in blk.instructions:
                si = getattr(ins, "sync_info", None)
                if si is not None and si.on_wait and len(si.on_wait) > max_waits:
                    waits = list(si.on_wait)
                    si.on_wait = waits[-max_waits:]
                    for i, w in enumerate(waits[:-max_waits]):
                        new.append(
                            mybir.InstNoOp(
                                name=f"{ins.name}-waitfix-{i}",
                                engine=ins.engine,
                                sync_info=mybir.SyncInfo(on_wait=[w], on_update=[]),
                                bass_nofuse=True,
                            )
                        )
                        n_fix += 1
                new.append(ins)
            blk.instructions[:] = new
    return n_fix


def build_nc(n_steps):
    """Build the per-core SPMD program. n_steps = S - 1 recurrent steps.

    Two time-chains run interleaved to hide the per-step serial-dependency
    latency: chain A computes h_1..h_KA exactly; chain B computes
    h_{KA+1}..h_{n_steps}, warm-started WARM steps early from h=0 (the GRU
    contracts ~0.8/step, so the warm-start error is ~1e-7 by its first real
    step). B's warmup consumes private copies of its input blocks so its
    approximate xhat writes don't pollute chain A's inputs."""
    nc = bass.Bass()
    dt = mybir.dt
    n_chunks = (n_steps + CH - 1) // CH
    two_chain = n_steps >= 3 * WARM
    if two_chain:
        KA = (n_steps + 1 + WARM) // 2
        BN = WARM + n_steps - KA
    else:
        KA, BN = n_steps, 0

    # xm rows 0:64 = x values (block 0 pre-imputed on host), 64:128 = 1-m
    xm = nc.dram_tensor("xm", [128, n_steps, NB], dt.bfloat16, kind="ExternalInput")
    wih = nc.dram_tensor("wih", [2 * F, 3 * H], dt.bfloat16, kind="ExternalInput")
    whh = nc.dram_tensor("whh", [128, NFOLD * 3 * H], dt.bfloat16, kind="ExternalInput")
    # stacked readout: fold c -> [Wro.T fold | WoutX.T fold] = [128, 128]
    wro = nc.dram_tensor("wro", [128, NFOLD * 128], dt.bfloat16, kind="ExternalInput")
    # bst cols: 0:128 r-bias folds, 128:256 z, 256:384 gi_n, 384:512 gh_n
    # rows 0:4 = bf16-high component per fold, rows 4:8 = bf16-low remainder
    bst = nc.dram_tensor("bst", [8, 4 * 128], dt.bfloat16, kind="ExternalInput")
    ind8 = nc.dram_tensor("ind8", [8, 128], dt.bfloat16, kind="ExternalInput")
    # brop: rows 0:2 = bro hi/lo (cols 0:64), zero elsewhere; ones [2, NB]
    brop = nc.dram_tensor("brop", [2, 128], dt.bfloat16, kind="ExternalInput")
    ones = nc.dram_tensor("ones", [2, NB], dt.bfloat16, kind="ExternalInput")
    if two_chain:
        # private warmup copy of blocks KA-WARM..KA-1 (block 0 pre-imputed)
        xw = nc.dram_tensor("xw", [128, WARM, NB], dt.bfloat16,
                            kind="ExternalInput")

    op_out = nc.dram_tensor("op", [128, n_steps, NB], dt.float32, kind="ExternalOutput")

    with tile.TileContext(nc) as tc:
        with (
            tc.tile_pool(name="singles", bufs=1) as singles,
            tc.tile_pool(name="hist", bufs=1) as hist,
            tc.tile_pool(name="work", bufs=2) as work,
            tc.tile_pool(name="ps", bufs=1, space="PSUM") as psp,
            tc.tile_pool(name="outs", bufs=3) as outs,
        ):
            # --- load weights / biases (once) ---
            wih_sb = singles.tile([2 * F, 3 * H], dt.bfloat16)
            nc.sync.dma_start(out=wih_sb, in_=wih[:])
            whh_sb = singles.tile([128, NFOLD * 3 * H], dt.bfloat16)
            nc.sync.dma_start(out=whh_sb, in_=whh[:])
            wro_sb = singles.tile([128, NFOLD * 128], dt.bfloat16)
            nc.sync.dma_start(out=wro_sb, in_=wro[:])
            bst_sb = singles.tile([8, 4 * 128], dt.bfloat16)
            nc.sync.dma_start(out=bst_sb, in_=bst[:])
            ind8_sb = singles.tile([8, 128], dt.bfloat16)
            nc.sync.dma_start(out=ind8_sb, in_=ind8[:])
            brop_sb = singles.tile([2, 128], dt.bfloat16)
            nc.sync.dma_start(out=brop_sb, in_=brop[:])
            ones_sb = singles.tile([2, NB], dt.bfloat16)
            nc.sync.dma_start(out=ones_sb, in_=ones[:])

            # --- resident input, chunked so chunk 0 gates only early steps.
            # mch duplicates the 1-m rows at partitions 0:64 because
            # copy_predicated needs out/mask/data partition-aligned. ---
            xch, mch = [], []
            for c in range(n_chunks):
                c0 = c * CH
                c1 = min(n_steps, c0 + CH)
                xt = singles.tile([128, c1 - c0, NB], dt.bfloat16, name=f"xch{c}")
                nc.sync.dma_start(out=xt, in_=xm[:, c0:c1, :])
                xch.append(xt)
                mt = singles.tile([F, c1 - c0, NB], dt.bfloat16, name=f"mch{c}")
                nc.sync.dma_start(out=mt, in_=xm[F : 2 * F, c0:c1, :])
                mch.append(mt)

            def xblk(b):
                return xch[b // CH][:, b % CH, :]

            def mblk(b):
                return mch[b // CH][:, b % CH, :]

            if two_chain:
                xw_sb = singles.tile([128, WARM, NB], dt.bfloat16)
                nc.sync.dma_start(out=xw_sb, in_=xw[:])

            def whh_sl(c2, gs):
                base = c2 * 3 * H + 128 * gs
                return whh_sb[:, base : base + 128]

            def wih_sl(gs):
                return wih_sb[:, 128 * gs : 128 * (gs + 1)]

            class Chain:
                pass

            cha = Chain()
            cha.sfx, cha.n = "A", KA
            cha.blk = lambda i: i - 1
            cha.xin = lambda i: xblk(i - 1)
            cha.out_j = lambda i: i - 2
            chains = [cha]
            if two_chain:
                chb = Chain()
                chb.sfx, chb.n = "B", BN
                chb.blk = lambda i: KA - WARM + i - 1
                chb.xin = lambda i: (xw_sb[:, i - 1, :] if i <= WARM
                                     else xblk(KA - WARM + i - 1))
                chb.out_j = lambda i: (KA - WARM + i - 2
                                       if KA - WARM + i - 2 >= KA else None)
                chains.append(chb)

            for c in chains:
                # hidden state ring: [128, parity, fold*NB]; parity = t % 2
                c.h = hist.tile([128, 2, NFOLD * NB], dt.bfloat16,
                                name=f"h{c.sfx}")
                nc.vector.memset(c.h[:, 0, :], 0.0)
                c.hfold = (lambda cc: lambda pv, c2:
                           cc.h[:, pv, c2 * NB : (c2 + 1) * NB])(c)
                # zero "order token": rewritten (as 0) from n_t at the end of
                # each phase2; the NEXT phase2's sigmoid reads it as bias so
                # the scheduler cannot hoist that sigmoid ahead of this
                # chain's tanh (prevents scalar-FIFO priority inversions).
                c.tok = hist.tile([128, 1], dt.float32, name=f"tok{c.sfx}")
                nc.vector.memset(c.tok, 0.0)

            def emit_readout(c, i, pv, tail=False):
                """Readout of h_{i-1} (or h_n for tail): psum <- bro +
                [Wro|WoutX]^T h (bro via K=2 seed matmul); predicated xhat
                overwrite into x block straight from PSUM. The SBUF copy +
                DMA for exact outputs is deferred to phase2 (off the
                critical path). Returns ps_ro."""
                ps_ro = psp.tile([128, NB], dt.float32, tag="ro", bufs=2,
                                 padded_shape=[128, 512], name=f"ro{c.sfx}{i}")
                nc.tensor.matmul(ps_ro, brop_sb, ones_sb, start=True, stop=False,
                                 skip_group_check=True)
                for c2 in range(NFOLD):
                    nc.tensor.matmul(ps_ro, wro_sb[:, c2 * 128 : (c2 + 1) * 128],
                                     c.hfold(pv, c2), start=False,
                                     stop=(c2 == NFOLD - 1),
                                     skip_group_check=True)
                if not tail:
                    nc.vector.copy_predicated(
                        c.xin(i)[0:F, :],
                        mblk(c.blk(i)).bitcast(mybir.dt.uint16),
                        ps_ro[0:F, :],
                    )
                return ps_ro

            def emit_out(c, i, ps_ro, tail=False):
                out_j = (c.n - 1 + (KA - WARM if c.sfx == "B" else 0)) if tail \
                    else c.out_j(i)
                if out_j is not None:
                    out_t = outs.tile([128, NB], dt.float32, tag="out_t",
                                      name=f"out{c.sfx}{i}")
                    nc.scalar.activation(out=out_t, in_=ps_ro, func=AF.Copy)
                    nc.sync.dma_start(out=op_out[:, out_j, :], in_=out_t)

            def phase1(c, i):
                """Readout + predicated-impute + the full matmul stream."""
                pv = (i - 1) % 2
                ps_ro = emit_readout(c, i, pv) if i >= 2 else None
                x_in = c.xin(i)

                # per-chain banks so start=True seeds never WAR on the other
                # chain's readers (head-of-line stall in the tensor FIFO);
                # Ngh+Ngi share one bank (regions 0:128 / 128:256).
                rbk = psp.tile([128, NFOLD * NB], dt.float32, tag=f"rbank{c.sfx}",
                               padded_shape=[128, 512], name=f"rb{c.sfx}{i}")
                zbk = psp.tile([128, NFOLD * NB], dt.float32, tag=f"zbank{c.sfx}",
                               padded_shape=[128, 512], name=f"zb{c.sfx}{i}")
                nnb = psp.tile([128, 2 * NFOLD * NB], dt.float32, tag=f"nnbank{c.sfx}",
                               padded_shape=[128, 512], name=f"nn{c.sfx}{i}")
                ngh = nnb[:, 0 : NFOLD * NB]
                ngi = nnb[:, NFOLD * NB : 2 * NFOLD * NB]

                def seed(reg, bias_col, start):
                    nc.tensor.matmul(
                        reg, bst_sb[:, bias_col * 128 : (bias_col + 1) * 128],
                        ind8_sb[:, 0 : NFOLD * NB],
                        start=start, stop=False, skip_group_check=True,
                    )

                def bank_mms(bk, bias_col, gs0, with_gi, last_stop):
                    seed(bk[:, 0 : NFOLD * NB], bias_col, start=True)
                    for s in range(NFOLD):
                        reg = bk[:, s * NB : (s + 1) * NB]
                        for c2 in range(NFOLD):
                            nc.tensor.matmul(
                                reg, whh_sl(c2, gs0 + s), c.hfold(pv, c2),
                                start=False,
                                stop=(last_stop and not with_gi
                                      and s == NFOLD - 1 and c2 == NFOLD - 1),
                                skip_group_check=True,
                            )
                    if with_gi:
                        for s in range(NFOLD):
                            reg = bk[:, s * NB : (s + 1) * NB]
                            nc.tensor.matmul(
                                reg, wih_sl(gs0 + s), x_in,
                                start=False, stop=(last_stop and s == NFOLD - 1),
                                skip_group_check=True,
                            )

                # tensor stream: R -> N (gh region + gi region) -> Z(last)
                bank_mms(rbk, 0, 0, with_gi=True, last_stop=True)
                bank_mms(nnb, 3, 8, with_gi=False, last_stop=False)
                seed(ngi, 2, start=False)
                for s in range(NFOLD):
                    nc.tensor.matmul(
                        ngi[:, s * NB : (s + 1) * NB], wih_sl(8 + s), x_in,
                        start=False, stop=(s == NFOLD - 1),
                        skip_group_check=True,
                    )
                bank_mms(zbk, 1, 4, with_gi=True, last_stop=True)
                c.cur = (i, ps_ro, rbk, zbk, ngh, ngi)

            def phase2(c):
                """Gate nonlinearities + state update + deferred output."""
                i, ps_ro, rbk, zbk, ngh, ngi = c.cur
                pv, cur = (i - 1) % 2, i % 2
                r_t = work.tile([128, NFOLD * NB], dt.bfloat16,
                                tag=f"r_t{c.sfx}", name=f"r{c.sfx}{i}")
                nc.scalar.activation(out=r_t, in_=rbk, func=AF.Sigmoid,
                                     bias=self_state["prev_tok"][:, 0:1])
                nin1 = work.tile([128, NFOLD * NB], dt.float32,
                                 tag=f"nin1{c.sfx}", name=f"n1{c.sfx}{i}")
                nc.vector.tensor_tensor(nin1, ngh, r_t, ALU.mult)
                nin2 = work.tile([128, NFOLD * NB], dt.float32,
                                 tag=f"nin2{c.sfx}", name=f"n2{c.sfx}{i}")
                nc.vector.tensor_tensor(nin2, nin1, ngi, ALU.add)

                z_t = work.tile([128, NFOLD * NB], dt.bfloat16,
                                tag=f"z_t{c.sfx}", name=f"z{c.sfx}{i}")
                nc.scalar.activation(out=z_t, in_=zbk, func=AF.Sigmoid)
                n_t = work.tile([128, NFOLD * NB], dt.bfloat16,
                                tag=f"n_t{c.sfx}", name=f"n{c.sfx}{i}")
                nc.scalar.activation(out=n_t, in_=nin2, func=AF.Tanh)

                omz = work.tile([128, NFOLD * NB], dt.bfloat16,
                                tag=f"omz{c.sfx}", name=f"om{c.sfx}{i}")
                nc.gpsimd.tensor_scalar(omz, z_t, -1.0, 1.0, ALU.mult, ALU.add)
                zh = work.tile([128, NFOLD * NB], dt.bfloat16,
                               tag=f"zh{c.sfx}", name=f"zh{c.sfx}{i}")
                nc.gpsimd.tensor_tensor(zh, z_t, c.h[:, pv, :], ALU.mult)
                t3 = work.tile([128, NFOLD * NB], dt.bfloat16,
                               tag=f"t3{c.sfx}", name=f"t3{c.sfx}{i}")
                nc.vector.tensor_tensor(t3, n_t, omz, ALU.mult)
                nc.vector.tensor_tensor(c.h[:, cur, :], t3, zh, ALU.add)
                nc.vector.tensor_scalar(c.tok, n_t[:, 0:1], 0.0, None, ALU.mult)
                self_state["prev_tok"] = c.tok
                if ps_ro is not None:
                    emit_out(c, i, ps_ro)

            # software-pipelined emission: each chain's matmul stream is
            # emitted between the other chain's phase1 and phase2, so the
            # per-engine FIFO order matches the intended interleaved
            # schedule (A-stream || B-chain, then B-stream || A-chain).
            self_state = {"prev_tok": chains[-1].tok}
            cb = chains[1] if two_chain else None
            for it in range(1, cha.n + 1):
                phase1(cha, it)
                if cb is not None and it >= 2 and it - 1 <= cb.n:
                    phase2(cb)       # covers all of B: BN <= KA-1
                if cb is not None and it <= cb.n:
                    phase1(cb, it)
                phase2(cha)
            for c in chains:
                ps_ro = emit_readout(c, c.n + 1, c.n % 2, tail=True)
                emit_out(c, c.n + 1, ps_ro, tail=True)

    _legalize_multiwait(nc)
    return nc


_NC_CACHE = {}


def _get_nc(n_steps):
    if n_steps not in _NC_CACHE:
        _NC_CACHE[n_steps] = build_nc(n_steps)
    return _NC_CACHE[n_steps]


def _prep_core_inputs(x2d, m2d, Wih, Whh, bih, bhh, Wro, bro, Wout_half, n_steps):
    """Per-core input map. x2d/m2d: [NB, S_loc, F] float32/bool already
    direction-ordered (time-reversed for backward cores)."""
    Wih = np.asarray(Wih, np.float32)
    bih = np.asarray(bih, np.float32)
    bhh = np.asarray(bhh, np.float32)
    bro_f = np.asarray(bro, np.float32)

    xt = np.ascontiguousarray(x2d[:, :n_steps].transpose(2, 1, 0)).astype(np.float32)
    mt = m2d[:, :n_steps].transpose(2, 1, 0)          # [F, t, NB] bool
    # block 0 x-rows pre-imputed with bro (xhat_0); mask rows hold 1-m
    xt[:, 0, :] = np.where(mt[:, 0, :], xt[:, 0, :], bro_f[:, None])
    xm = np.concatenate([xt, 1.0 - mt.astype(np.float32)], axis=0).astype(BF16)

    extra = {}
    if n_steps >= 3 * WARM:
        ka = (n_steps + 1 + WARM) // 2
        xw_f = xt[:, ka - WARM : ka].copy()           # [F, WARM, NB]
        mw = mt[:, ka - WARM : ka]
        xw_f[:, 0] = np.where(mw[:, 0], xw_f[:, 0], bro_f[:, None])
        extra["xw"] = np.concatenate(
            [xw_f, 1.0 - mw.astype(np.float32)], axis=0).astype(BF16)

    wih_t = Wih.T.copy()                               # [2F, 3H]
    wih_t[F:] = -wih_t[F:]                             # mask half negated
    wih_t = np.ascontiguousarray(wih_t).astype(BF16)
    whh_t = np.ascontiguousarray(
        np.asarray(Whh, np.float32).T.reshape(NFOLD, 128, 3 * H)
        .transpose(1, 0, 2).reshape(128, NFOLD * 3 * H)
    ).astype(BF16)
    wro_f = np.asarray(Wro, np.float32).T.reshape(NFOLD, 128, F)
    wout_f = np.asarray(Wout_half, np.float32).T.reshape(NFOLD, 128, F)
    wro_t = np.ascontiguousarray(
        np.concatenate([wro_f, wout_f], axis=2)
        .transpose(1, 0, 2).reshape(128, NFOLD * 128)
    ).astype(BF16)

    # biases with the mask-rowsum adjustment (m = 1 - inv_m)
    radj = Wih[:, F:].sum(axis=1)                      # [3H]
    bsum = bih + bhh + radj
    b_r, b_z = bsum[0:H], bsum[H : 2 * H]
    b_in = bih[2 * H :] + radj[2 * H :]
    b_hn = bhh[2 * H :]
    bst_f = np.empty((4, 4 * 128), np.float32)
    for k in range(4):
        bst_f[k, 0:128] = b_r[128 * k : 128 * (k + 1)]
        bst_f[k, 128:256] = b_z[128 * k : 128 * (k + 1)]
        bst_f[k, 256:384] = b_in[128 * k : 128 * (k + 1)]
        bst_f[k, 384:512] = b_hn[128 * k : 128 * (k + 1)]
    # hi/lo bf16 split: rows 0:4 = bf16(b), rows 4:8 = bf16(b - hi)
    bst = np.empty((8, 4 * 128), BF16)
    bst[0:4] = bst_f.astype(BF16)
    bst[4:8] = (bst_f - bst[0:4].astype(np.float32)).astype(BF16)
    ind8 = np.zeros((8, 128), np.float32)
    for k in range(4):
        ind8[k, 32 * k : 32 * (k + 1)] = 1.0
        ind8[4 + k, 32 * k : 32 * (k + 1)] = 1.0
    brop_f = np.zeros((2, 128), np.float32)
    brop_f[0, 0:F] = bro_f
    brop = np.empty((2, 128), BF16)
    brop[0] = brop_f[0].astype(BF16)
    brop[1] = (brop_f[0] - brop[0].astype(np.float32)).astype(BF16)

    return {
        "xm": xm, "wih": wih_t, "whh": whh_t, "wro": wro_t,
        "bst": bst, "ind8": ind8.astype(BF16), "brop": brop,
        "ones": np.ones((2, NB), BF16), **extra,
    }


def run_device(inputs, s_len=S, trace=False):
    """Run the 8-core SPMD kernel. Returns BassKernelResults."""
    n_steps = s_len - 1
    nc = _get_nc(n_steps)

    x2d = np.asarray(inputs["x"], np.float32).reshape(B, S, F)[:, :s_len]
    m2d = np.asarray(inputs["mask"]).reshape(B, S, F)[:, :s_len]

    in_maps = []
    for core in range(8):
        g = core % 4
        bsl = slice(NB * g, NB * (g + 1))
        if core < 4:
            im = _prep_core_inputs(
                x2d[bsl], m2d[bsl], inputs["Wih_f"], inputs["Whh_f"],
                inputs["bih_f"], inputs["bhh_f"], inputs["Wro_f"], inputs["bro_f"],
                np.asarray(inputs["Wout"])[:, :H], n_steps,
            )
        else:
            im = _prep_core_inputs(
                x2d[bsl, ::-1], m2d[bsl, ::-1], inputs["Wih_b"], inputs["Whh_b"],
                inputs["bih_b"], inputs["bhh_b"], inputs["Wro_b"], inputs["bro_b"],
                np.asarray(inputs["Wout"])[:, H:], n_steps,
            )
        in_maps.append(im)

    return run_bass_kernel_spmd(nc, in_maps, core_ids=list(range(8)), trace=trace)


def assemble(inputs, res, s_len=S):
    """Host-side gather: combine per-core outputs into full reference outputs."""
    n_steps = s_len - 1
    bro_f = np.asarray(inputs["bro_f"], np.float32)
    bro_b = np.asarray(inputs["bro_b"], np.float32)
    bout = np.asarray(inputs["bout"], np.float32)

    xh_f = np.empty((B, s_len, F), np.float32)
    xh_b = np.empty((B, s_len, F), np.float32)
    x_hat = np.empty((B, s_len, F), np.float32)

    for g in range(4):
        bsl = slice(NB * g, NB * (g + 1))
        rf, rb = res.results[g], res.results[g + 4]
        # device output "op" is [128, n_steps, NB]: rows 0:64 xhat, 64:128 pp
        xh_f[bsl, 1:] = rf["op"][:F].transpose(2, 1, 0)
        xh_f[bsl, 0] = bro_f
        xh_b[bsl, :n_steps] = rb["op"][:F].transpose(2, 1, 0)[:, ::-1]
        xh_b[bsl, n_steps] = bro_b
        pf = rf["op"][F:].transpose(2, 1, 0)
        pb = rb["op"][F:].transpose(2, 1, 0)[:, ::-1]
        x_hat[bsl, 1:] = pf
        x_hat[bsl, 0] = 0.0
        x_hat[bsl, :n_steps] += pb
        x_hat[bsl] += bout

    return (
        x_hat.reshape(B, s_len, N, C),
        xh_f.reshape(B, s_len, N, C),
        xh_b.reshape(B, s_len, N, C),
    )


def kernel(**inputs):
    res = run_device(inputs, s_len=S)
    return assemble(inputs, res, s_len=S)


# revision 28
# speedup vs baseline: 1.2052x; 1.2052x over previous
"""Trainium2 Bass kernel for nn_BiRNNImputerModel (bidirectional GRU imputer).

Strategy (v2 — fold-batched gate math):
  - 8 cores: cores 0-3 run the forward GRU, cores 4-7 the backward GRU
    (backward = same program on time-reversed inputs).
  - Within a direction, data-parallel over batch: 128 / 4 = 32 per core.
  - On-chip layout is "transposed" [feature/H, batch] so recurrent matmuls
    need no per-step transposes; the 4 H-folds (512 = 4*128) of each gate
    live in the FREE dim of one PSUM bank: bank = [128, 4 folds * 32 batch].
    Gate nonlinearities then run as ONE [128,128] ACTIVATE per gate instead
    of 4 x [128,32] — the scalar/vector fixed per-instruction overhead
    (~300ns) dominated the old per-fold version.
  - Per-fold gate biases can't ride the ACTIVATE bias port (bias varies
    along the free dim), so each gate bank is seeded by a tiny K=4
    "indicator" matmul (stationary = 4 stacked fold-biases, moving = 0/1
    fold indicator) as the bank's start=True first write.
  - Input x/mask are SBUF-resident for the whole sequence, stored as
    [x ; 1-m] with the mask-half of Wih negated and sum_f Wih_m[:,f] folded
    into the biases. The per-step imputation x_p = m ? x : xhat is then a
    single copy_predicated that overwrites x in place (pred = 1-m) with
    xhat; the resident column block IS the gi matmul moving operand.
  - The per-step readout matmul uses a stacked stationary [Wro.T | WoutX.T]
    producing xhat_t and this direction's partial of the final
    bidirectional readout in one accumulation. bro rides the output-copy
    ACTIVATE's per-partition bias port.
  - Cross-direction sum + bout + layout fixes happen on the host; no
    cross-core communication.

PSUM discipline: every PSUM tile is padded to a full 2KB bank. Each bank
gets exactly ONE start=True per step (its seeding bias matmul / first
readout matmul); all other matmuls use start=False and rely on per-element
has_written accumulate-or-overwrite semantics.
"""

import os
import sys

for _p in ("/opt/trn_rl_repo", "/root/.axon_site/_ro/trn_rl_repo"):
    if os.path.isdir(_p) and _p not in sys.path:
        sys.path.insert(0, _p)

import numpy as np
import ml_dtypes

import concourse.bass as bass
import concourse.tile as tile
from concourse import mybir
from concourse.bass_utils import run_bass_kernel_spmd

BF16 = ml_dtypes.bfloat16

B, S, N, C = 128, 512, 64, 1
F = N * C          # 64
H = 512
NB = 32            # batch per core (128 / 4)
NFOLD = 4          # H / 128
CH = 64            # steps per resident-input chunk
WARM = 64          # chain-B warm-start steps (state error decays ~0.8^WARM)
AF = mybir.ActivationFunctionType
ALU = mybir.AluOpType


def _legalize_multiwait(nc, max_waits=1):
    """walrus in this image only encodes one sync-wait per instruction;
    hoist extra waits onto preceding NoOps."""
    n_fix = 0
    for f in nc.m.functions:
        for blk in f.blocks:
            new = []
            for ins in blk.instructions:
                si = getattr(ins, "sync_info", None)
                if si is not None and si.on_wait and len(si.on_wait) > max_waits:
                    waits = list(si.on_wait)
                    si.on_wait = waits[-max_waits:]
                    for i, w in enumerate(waits[:-max_waits]):
                        new.append(
                            mybir.InstNoOp(
                                name=f"{ins.name}-waitfix-{i}",
                                engine=ins.engine,
                                sync_info=mybir.SyncInfo(on_wait=[w], on_update=[]),
                                bass_nofuse=True,
                            )
                        )
                        n_fix += 1
                new.append(ins)
            blk.instructions[:] = new
    return n_fix


def build_nc(n_steps):
    """Build the per-core SPMD program. n_steps = S - 1 recurrent steps.

    Two time-chains run interleaved to hide the per-step serial-dependency
    latency: chain A computes h_1..h_KA exactly; chain B computes
    h_{KA+1}..h_{n_steps}, warm-started WARM steps early from h=0 (the GRU
    contracts ~0.8/step, so the warm-start error is ~1e-7 by its first real
    step). B's warmup consumes private copies of its input blocks so its
    approximate xhat writes don't pollute chain A's inputs."""
    nc = bass.Bass()
    dt = mybir.dt
    n_chunks = (n_steps + CH - 1) // CH
    two_chain = n_steps >= 3 * WARM
    if two_chain:
        KA = (n_steps + 1 + WARM) // 2
        BN = WARM + n_steps - KA
    else:
        KA, BN = n_steps, 0

    # xm rows 0:64 = x values (block 0 pre-imputed on host), 64:128 = 1-m
    xm = nc.dram_tensor("xm", [128, n_steps, NB], dt.bfloat16, kind="ExternalInput")
    wih = nc.dram_tensor("wih", [2 * F, 3 * H], dt.bfloat16, kind="ExternalInput")
    whh = nc.dram_tensor("whh", [128, NFOLD * 3 * H], dt.bfloat16, kind="ExternalInput")
    # stacked readout: fold c -> [Wro.T fold | WoutX.T fold] = [128, 128]
    wro = nc.dram_tensor("wro", [128, NFOLD * 128], dt.bfloat16, kind="ExternalInput")
    # bst cols: 0:128 r-bias folds, 128:256 z, 256:384 gi_n, 384:512 gh_n
    # rows 0:4 = bf16-high component per fold, rows 4:8 = bf16-low remainder
    bst = nc.dram_tensor("bst", [8, 4 * 128], dt.bfloat16, kind="ExternalInput")
    ind8 = nc.dram_tensor("ind8", [8, 128], dt.bfloat16, kind="ExternalInput")
    # brop: rows 0:2 = bro hi/lo (cols 0:64), zero elsewhere; ones [2, NB]
    brop = nc.dram_tensor("brop", [2, 128], dt.bfloat16, kind="ExternalInput")
    ones = nc.dram_tensor("ones", [2, NB], dt.bfloat16, kind="ExternalInput")
    if two_chain:
        # private warmup copy of blocks KA-WARM..KA-1 (block 0 pre-imputed)
        xw = nc.dram_tensor("xw", [128, WARM, NB], dt.bfloat16,
                            kind="ExternalInput")

    op_out = nc.dram_tensor("op", [128, n_steps, NB], dt.float32, kind="ExternalOutput")

    with tile.TileContext(nc) as tc:
        with (
            tc.tile_pool(name="singles", bufs=1) as singles,
            tc.tile_pool(name="hist", bufs=1) as hist,
            tc.tile_pool(name="work", bufs=2) as work,
            tc.tile_pool(name="ps", bufs=1, space="PSUM") as psp,
            tc.tile_pool(name="outs", bufs=3) as outs,
        ):
            # --- load weights / biases (once) ---
            wih_sb = singles.tile([2 * F, 3 * H], dt.bfloat16)
            nc.sync.dma_start(out=wih_sb, in_=wih[:])
            whh_sb = singles.tile([128, NFOLD * 3 * H], dt.bfloat16)
            nc.sync.dma_start(out=whh_sb, in_=whh[:])
            wro_sb = singles.tile([128, NFOLD * 128], dt.bfloat16)
            nc.sync.dma_start(out=wro_sb, in_=wro[:])
            bst_sb = singles.tile([8, 4 * 128], dt.bfloat16)
            nc.sync.dma_start(out=bst_sb, in_=bst[:])
            ind8_sb = singles.tile([8, 128], dt.bfloat16)
            nc.sync.dma_start(out=ind8_sb, in_=ind8[:])
            brop_sb = singles.tile([2, 128], dt.bfloat16)
            nc.sync.dma_start(out=brop_sb, in_=brop[:])
            ones_sb = singles.tile([2, NB], dt.bfloat16)
            nc.sync.dma_start(out=ones_sb, in_=ones[:])

            # --- resident input, chunked so chunk 0 gates only early steps.
            # mch duplicates the 1-m rows at partitions 0:64 because
            # copy_predicated needs out/mask/data partition-aligned. ---
            xch, mch = [], []
            for c in range(n_chunks):
                c0 = c * CH
                c1 = min(n_steps, c0 + CH)
                xt = singles.tile([128, c1 - c0, NB], dt.bfloat16, name=f"xch{c}")
                nc.sync.dma_start(out=xt, in_=xm[:, c0:c1, :])
                xch.append(xt)
                mt = singles.tile([F, c1 - c0, NB], dt.bfloat16, name=f"mch{c}")
                nc.sync.dma_start(out=mt, in_=xm[F : 2 * F, c0:c1, :])
                mch.append(mt)

            def xblk(b):
                return xch[b // CH][:, b % CH, :]

            def mblk(b):
                return mch[b // CH][:, b % CH, :]

            if two_chain:
                xw_sb = singles.tile([128, WARM, NB], dt.bfloat16)
                nc.sync.dma_start(out=xw_sb, in_=xw[:])

            def whh_sl(c2, gs):
                base = c2 * 3 * H + 128 * gs
                return whh_sb[:, base : base + 128]

            def wih_sl(gs):
                return wih_sb[:, 128 * gs : 128 * (gs + 1)]

            class Chain:
                pass

            cha = Chain()
            cha.sfx, cha.n = "A", KA
            cha.blk = lambda i: i - 1
            cha.xin = lambda i: xblk(i - 1)
            cha.out_j = lambda i: i - 2
            chains = [cha]
            if two_chain:
                chb = Chain()
                chb.sfx, chb.n = "B", BN
                chb.blk = lambda i: KA - WARM + i - 1
                chb.xin = lambda i: (xw_sb[:, i - 1, :] if i <= WARM
                                     else xblk(KA - WARM + i - 1))
                chb.out_j = lambda i: (KA - WARM + i - 2
                                       if KA - WARM + i - 2 >= KA else None)
                chains.append(chb)

            for c in chains:
                # hidden state ring: [128, parity, fold*NB]; parity = t % 2
                c.h = hist.tile([128, 2, NFOLD * NB], dt.bfloat16,
                                name=f"h{c.sfx}")
                nc.vector.memset(c.h[:, 0, :], 0.0)
                c.hfold = (lambda cc: lambda pv, c2:
                           cc.h[:, pv, c2 * NB : (c2 + 1) * NB])(c)
                # zero "order token": rewritten (as 0) from n_t at the end of
                # each phase2; the NEXT phase2's sigmoid reads it as bias so
                # the scheduler cannot hoist that sigmoid ahead of this
                # chain's tanh (prevents scalar-FIFO priority inversions).
                c.tok = hist.tile([128, 1], dt.float32, name=f"tok{c.sfx}")
                nc.vector.memset(c.tok, 0.0)

            def emit_readout(c, i, pv, tail=False):
                """Readout of h_{i-1} (or h_n for tail): psum <- bro +
                [Wro|WoutX]^T h (bro via K=2 seed matmul); predicated xhat
                overwrite into x block straight from PSUM. The SBUF copy +
                DMA for exact outputs is deferred to phase2 (off the
                critical path). Returns ps_ro."""
                ps_ro = psp.tile([128, NB], dt.float32, tag="ro", bufs=2,
                                 padded_shape=[128, 512], name=f"ro{c.sfx}{i}")
                nc.tensor.matmul(ps_ro, brop_sb, ones_sb, start=True, stop=False,
                                 skip_group_check=True)
                for c2 in range(NFOLD):
                    nc.tensor.matmul(ps_ro, wro_sb[:, c2 * 128 : (c2 + 1) * 128],
                                     c.hfold(pv, c2), start=False,
                                     stop=(c2 == NFOLD - 1),
                                     skip_group_check=True)
                if not tail:
                    nc.vector.copy_predicated(
                        c.xin(i)[0:F, :],
                        mblk(c.blk(i)).bitcast(mybir.dt.uint16),
                        ps_ro[0:F, :],
                    )
                return ps_ro

            def emit_out(c, i, ps_ro, tail=False):
                out_j = (c.n - 1 + (KA - WARM if c.sfx == "B" else 0)) if tail \
                    else c.out_j(i)
                if out_j is not None:
                    out_t = outs.tile([128, NB], dt.float32, tag="out_t",
                                      name=f"out{c.sfx}{i}")
                    nc.scalar.activation(out=out_t, in_=ps_ro, func=AF.Copy)
                    nc.sync.dma_start(out=op_out[:, out_j, :], in_=out_t)

            def phase1(c, i):
                """Readout + predicated-impute + the full matmul stream."""
                pv = (i - 1) % 2
                ps_ro = emit_readout(c, i, pv) if i >= 2 else None
                x_in = c.xin(i)

                # per-chain banks so start=True seeds never WAR on the other
                # chain's readers (head-of-line stall in the tensor FIFO);
                # Ngh+Ngi share one bank (regions 0:128 / 128:256).
                rbk = psp.tile([128, NFOLD * NB], dt.float32, tag=f"rbank{c.sfx}",
                               padded_shape=[128, 512], name=f"rb{c.sfx}{i}")
                zbk = psp.tile([128, NFOLD * NB], dt.float32, tag=f"zbank{c.sfx}",
                               padded_shape=[128, 512], name=f"zb{c.sfx}{i}")
                nnb = psp.tile([128, 2 * NFOLD * NB], dt.float32, tag=f"nnbank{c.sfx}",
                               padded_shape=[128, 512], name=f"nn{c.sfx}{i}")
                ngh = nnb[:, 0 : NFOLD * NB]
                ngi = nnb[:, NFOLD * NB : 2 * NFOLD * NB]

                def seed(reg, bias_col, start):
                    nc.tensor.matmul(
                        reg, bst_sb[:, bias_col * 128 : (bias_col + 1) * 128],
                        ind8_sb[:, 0 : NFOLD * NB],
                        start=start, stop=False, skip_group_check=True,
                    )

                def bank_mms(bk, bias_col, gs0, with_gi, last_stop):
                    seed(bk[:, 0 : NFOLD * NB], bias_col, start=True)
                    for s in range(NFOLD):
                        reg = bk[:, s * NB : (s + 1) * NB]
                        for c2 in range(NFOLD):
                            nc.tensor.matmul(
                                reg, whh_sl(c2, gs0 + s), c.hfold(pv, c2),
                                start=False,
                                stop=(last_stop and not with_gi
                                      and s == NFOLD - 1 and c2 == NFOLD - 1),
                                skip_group_check=True,
                            )
                    if with_gi:
                        for s in range(NFOLD):
                            reg = bk[:, s * NB : (s + 1) * NB]
                            nc.tensor.matmul(
                                reg, wih_sl(gs0 + s), x_in,
                                start=False, stop=(last_stop and s == NFOLD - 1),
                                skip_group_check=True,
                            )

                # tensor stream: R -> N (gh region + gi region) -> Z(last)
                bank_mms(rbk, 0, 0, with_gi=True, last_stop=True)
                bank_mms(nnb, 3, 8, with_gi=False, last_stop=False)
                seed(ngi, 2, start=False)
                for s in range(NFOLD):
                    nc.tensor.matmul(
                        ngi[:, s * NB : (s + 1) * NB], wih_sl(8 + s), x_in,
                        start=False, stop=(s == NFOLD - 1),
                        skip_group_check=True,
                    )
                bank_mms(zbk, 1, 4, with_gi=True, last_stop=True)
                c.cur = (i, ps_ro, rbk, zbk, ngh, ngi)

            def phase2(c):
                """Gate nonlinearities + state update + deferred output."""
                i, ps_ro, rbk, zbk, ngh, ngi = c.cur
                pv, cur = (i - 1) % 2, i % 2
                r_t = work.tile([128, NFOLD * NB], dt.bfloat16,
                                tag=f"r_t{c.sfx}", name=f"r{c.sfx}{i}")
                nc.scalar.activation(out=r_t, in_=rbk, func=AF.Sigmoid,
                                     bias=self_state["prev_tok"][:, 0:1])
                nin1 = work.tile([128, NFOLD * NB], dt.float32,
                                 tag=f"nin1{c.sfx}", name=f"n1{c.sfx}{i}")
                nc.vector.tensor_tensor(nin1, ngh, r_t, ALU.mult)
                nin2 = work.tile([128, NFOLD * NB], dt.float32,
                                 tag=f"nin2{c.sfx}", name=f"n2{c.sfx}{i}")
                nc.vector.tensor_tensor(nin2, nin1, ngi, ALU.add)

                z_t = work.tile([128, NFOLD * NB], dt.bfloat16,
                                tag=f"z_t{c.sfx}", name=f"z{c.sfx}{i}")
                nc.scalar.activation(out=z_t, in_=zbk, func=AF.Sigmoid)
                n_t = work.tile([128, NFOLD * NB], dt.bfloat16,
                                tag=f"n_t{c.sfx}", name=f"n{c.sfx}{i}")
                nc.scalar.activation(out=n_t, in_=nin2, func=AF.Tanh)

                omz = work.tile([128, NFOLD * NB], dt.bfloat16,
                                tag=f"omz{c.sfx}", name=f"om{c.sfx}{i}")
                nc.gpsimd.tensor_scalar(omz, z_t, -1.0, 1.0, ALU.mult, ALU.add)
                zh = work.tile([128, NFOLD * NB], dt.bfloat16,
                               tag=f"zh{c.sfx}", name=f"zh{c.sfx}{i}")
                nc.gpsimd.tensor_tensor(zh, z_t, c.h[:, pv, :], ALU.mult)
                nc.gpsimd.tensor_scalar(c.tok, n_t[:, 0:1], 0.0, None, ALU.mult)
                t3 = work.tile([128, NFOLD * NB], dt.bfloat16,
                               tag=f"t3{c.sfx}", name=f"t3{c.sfx}{i}")
                nc.vector.tensor_tensor(t3, n_t, omz, ALU.mult)
                nc.vector.tensor_tensor(c.h[:, cur, :], t3, zh, ALU.add)
                self_state["prev_tok"] = c.tok
                if ps_ro is not None:
                    emit_out(c, i, ps_ro)

            # software-pipelined emission: each chain's matmul stream is
            # emitted between the other chain's phase1 and phase2, so the
            # per-engine FIFO order matches the intended interleaved
            # schedule (A-stream || B-chain, then B-stream || A-chain).
            self_state = {"prev_tok": chains[-1].tok}
            cb = chains[1] if two_chain else None
            for it in range(1, cha.n + 1):
                phase1(cha, it)
                if cb is not None and it >= 2 and it - 1 <= cb.n:
                    phase2(cb)       # covers all of B: BN <= KA-1
                if cb is not None and it <= cb.n:
                    phase1(cb, it)
                phase2(cha)
            for c in chains:
                ps_ro = emit_readout(c, c.n + 1, c.n % 2, tail=True)
                emit_out(c, c.n + 1, ps_ro, tail=True)

    _legalize_multiwait(nc)
    return nc


_NC_CACHE = {}


def _get_nc(n_steps):
    if n_steps not in _NC_CACHE:
        _NC_CACHE[n_steps] = build_nc(n_steps)
    return _NC_CACHE[n_steps]


def _prep_core_inputs(x2d, m2d, Wih, Whh, bih, bhh, Wro, bro, Wout_half, n_steps):
    """Per-core input map. x2d/m2d: [NB, S_loc, F] float32/bool already
    direction-ordered (time-reversed for backward cores)."""
    Wih = np.asarray(Wih, np.float32)
    bih = np.asarray(bih, np.float32)
    bhh = np.asarray(bhh, np.float32)
    bro_f = np.asarray(bro, np.float32)

    xt = np.ascontiguousarray(x2d[:, :n_steps].transpose(2, 1, 0)).astype(np.float32)
    mt = m2d[:, :n_steps].transpose(2, 1, 0)          # [F, t, NB] bool
    # block 0 x-rows pre-imputed with bro (xhat_0); mask rows hold 1-m
    xt[:, 0, :] = np.where(mt[:, 0, :], xt[:, 0, :], bro_f[:, None])
    xm = np.concatenate([xt, 1.0 - mt.astype(np.float32)], axis=0).astype(BF16)

    extra = {}
    if n_steps >= 3 * WARM:
        ka = (n_steps + 1 + WARM) // 2
        xw_f = xt[:, ka - WARM : ka].copy()           # [F, WARM, NB]
        mw = mt[:, ka - WARM : ka]
        xw_f[:, 0] = np.where(mw[:, 0], xw_f[:, 0], bro_f[:, None])
        extra["xw"] = np.concatenate(
            [xw_f, 1.0 - mw.astype(np.float32)], axis=0).astype(BF16)

    wih_t = Wih.T.copy()                               # [2F, 3H]
    wih_t[F:] = -wih_t[F:]                             # mask half negated
    wih_t = np.ascontiguousarray(wih_t).astype(BF16)
    whh_t = np.ascontiguousarray(
        np.asarray(Whh, np.float32).T.reshape(NFOLD, 128, 3 * H)
        .transpose(1, 0, 2).reshape(128, NFOLD * 3 * H)
    ).astype(BF16)
    wro_f = np.asarray(Wro, np.float32).T.reshape(NFOLD, 128, F)
    wout_f = np.asarray(Wout_half, np.float32).T.reshape(NFOLD, 128, F)
    wro_t = np.ascontiguousarray(
        np.concatenate([wro_f, wout_f], axis=2)
        .transpose(1, 0, 2).reshape(128, NFOLD * 128)
    ).astype(BF16)

    # biases with the mask-rowsum adjustment (m = 1 - inv_m)
    radj = Wih[:, F:].sum(axis=1)                      # [3H]
    bsum = bih + bhh + radj
    b_r, b_z = bsum[0:H], bsum[H : 2 * H]
    b_in = bih[2 * H :] + radj[2 * H :]
    b_hn = bhh[2 * H :]
    bst_f = np.empty((4, 4 * 128), np.float32)
    for k in range(4):
        bst_f[k, 0:128] = b_r[128 * k : 128 * (k + 1)]
        bst_f[k, 128:256] = b_z[128 * k : 128 * (k + 1)]
        bst_f[k, 256:384] = b_in[128 * k : 128 * (k + 1)]
        bst_f[k, 384:512] = b_hn[128 * k : 128 * (k + 1)]
    # hi/lo bf16 split: rows 0:4 = bf16(b), rows 4:8 = bf16(b - hi)
    bst = np.empty((8, 4 * 128), BF16)
    bst[0:4] = bst_f.astype(BF16)
    bst[4:8] = (bst_f - bst[0:4].astype(np.float32)).astype(BF16)
    ind8 = np.zeros((8, 128), np.float32)
    for k in range(4):
        ind8[k, 32 * k : 32 * (k + 1)] = 1.0
        ind8[4 + k, 32 * k : 32 * (k + 1)] = 1.0
    brop_f = np.zeros((2, 128), np.float32)
    brop_f[0, 0:F] = bro_f
    brop = np.empty((2, 128), BF16)
    brop[0] = brop_f[0].astype(BF16)
    brop[1] = (brop_f[0] - brop[0].astype(np.float32)).astype(BF16)

    return {
        "xm": xm, "wih": wih_t, "whh": whh_t, "wro": wro_t,
        "bst": bst, "ind8": ind8.astype(BF16), "brop": brop,
        "ones": np.ones((2, NB), BF16), **extra,
    }


def run_device(inputs, s_len=S, trace=False):
    """Run the 8-core SPMD kernel. Returns BassKernelResults."""
    n_steps = s_len - 1
    nc = _get_nc(n_steps)

    x2d = np.asarray(inputs["x"], np.float32).reshape(B, S, F)[:, :s_len]
    m2d = np.asarray(inputs["mask"]).reshape(B, S, F)[:, :s_len]

    in_maps = []
    for core in range(8):
        g = core % 4
        bsl = slice(NB * g, NB * (g + 1))
        if core < 4:
            im = _prep_core_inputs(
                x2d[bsl], m2d[bsl], inputs["Wih_f"], inputs["Whh_f"],
                inputs["bih_f"], inputs["bhh_f"], inputs["Wro_f"], inputs["bro_f"],
                np.asarray(inputs["Wout"])[:, :H], n_steps,
            )
        else:
            im = _prep_core_inputs(
                x2d[bsl, ::-1], m2d[bsl, ::-1], inputs["Wih_b"], inputs["Whh_b"],
                inputs["bih_b"], inputs["bhh_b"], inputs["Wro_b"], inputs["bro_b"],
                np.asarray(inputs["Wout"])[:, H:], n_steps,
            )
        in_maps.append(im)

    return run_bass_kernel_spmd(nc, in_maps, core_ids=list(range(8)), trace=trace)


def assemble(inputs, res, s_len=S):
    """Host-side gather: combine per-core outputs into full reference outputs."""
    n_steps = s_len - 1
    bro_f = np.asarray(inputs["bro_f"], np.float32)
    bro_b = np.asarray(inputs["bro_b"], np.float32)
    bout = np.asarray(inputs["bout"], np.float32)

    xh_f = np.empty((B, s_len, F), np.float32)
    xh_b = np.empty((B, s_len, F), np.float32)
    x_hat = np.empty((B, s_len, F), np.float32)

    for g in range(4):
        bsl = slice(NB * g, NB * (g + 1))
        rf, rb = res.results[g], res.results[g + 4]
        # device output "op" is [128, n_steps, NB]: rows 0:64 xhat, 64:128 pp
        xh_f[bsl, 1:] = rf["op"][:F].transpose(2, 1, 0)
        xh_f[bsl, 0] = bro_f
        xh_b[bsl, :n_steps] = rb["op"][:F].transpose(2, 1, 0)[:, ::-1]
        xh_b[bsl, n_steps] = bro_b
        pf = rf["op"][F:].transpose(2, 1, 0)
        pb = rb["op"][F:].transpose(2, 1, 0)[:, ::-1]
        x_hat[bsl, 1:] = pf
        x_hat[bsl, 0] = 0.0
        x_hat[bsl, :n_steps] += pb
        x_hat[bsl] += bout

    return (
        x_hat.reshape(B, s_len, N, C),
        xh_f.reshape(B, s_len, N, C),
        xh_b.reshape(B, s_len, N, C),
    )


def kernel(**inputs):
    res = run_device(inputs, s_len=S)
    return assemble(inputs, res, s_len=S)


# revision 33
# speedup vs baseline: 1.3416x; 1.1131x over previous
"""Trainium2 Bass kernel for nn_BiRNNImputerModel (bidirectional GRU imputer).

Strategy (v2 — fold-batched gate math):
  - 8 cores: cores 0-3 run the forward GRU, cores 4-7 the backward GRU
    (backward = same program on time-reversed inputs).
  - Within a direction, data-parallel over batch: 128 / 4 = 32 per core.
  - On-chip layout is "transposed" [feature/H, batch] so recurrent matmuls
    need no per-step transposes; the 4 H-folds (512 = 4*128) of each gate
    live in the FREE dim of one PSUM bank: bank = [128, 4 folds * 32 batch].
    Gate nonlinearities then run as ONE [128,128] ACTIVATE per gate instead
    of 4 x [128,32] — the scalar/vector fixed per-instruction overhead
    (~300ns) dominated the old per-fold version.
  - Per-fold gate biases can't ride the ACTIVATE bias port (bias varies
    along the free dim), so each gate bank is seeded by a tiny K=4
    "indicator" matmul (stationary = 4 stacked fold-biases, moving = 0/1
    fold indicator) as the bank's start=True first write.
  - Input x/mask are SBUF-resident for the whole sequence, stored as
    [x ; 1-m] with the mask-half of Wih negated and sum_f Wih_m[:,f] folded
    into the biases. The per-step imputation x_p = m ? x : xhat is then a
    single copy_predicated that overwrites x in place (pred = 1-m) with
    xhat; the resident column block IS the gi matmul moving operand.
  - The per-step readout matmul uses a stacked stationary [Wro.T | WoutX.T]
    producing xhat_t and this direction's partial of the final
    bidirectional readout in one accumulation. bro rides the output-copy
    ACTIVATE's per-partition bias port.
  - Cross-direction sum + bout + layout fixes happen on the host; no
    cross-core communication.

PSUM discipline: every PSUM tile is padded to a full 2KB bank. Each bank
gets exactly ONE start=True per step (its seeding bias matmul / first
readout matmul); all other matmuls use start=False and rely on per-element
has_written accumulate-or-overwrite semantics.
"""

import os
import sys

for _p in ("/opt/trn_rl_repo", "/root/.axon_site/_ro/trn_rl_repo"):
    if os.path.isdir(_p) and _p not in sys.path:
        sys.path.insert(0, _p)

import numpy as np
import ml_dtypes

import concourse.bass as bass
import concourse.tile as tile
from concourse import mybir
from concourse.bass_utils import run_bass_kernel_spmd

BF16 = ml_dtypes.bfloat16

B, S, N, C = 128, 512, 64, 1
F = N * C          # 64
H = 512
NB = 32            # batch per core (128 / 4)
NFOLD = 4          # H / 128
CH = 64            # steps per resident-input chunk
WARM = 64          # chain-B warm-start steps (state error decays ~0.8^WARM)
AF = mybir.ActivationFunctionType
ALU = mybir.AluOpType


def _legalize_multiwait(nc, max_waits=1):
    """walrus in this image only encodes one sync-wait per instruction;
    hoist extra waits onto preceding NoOps."""
    n_fix = 0
    for f in nc.m.functions:
        for blk in f.blocks:
            new = []
            for ins in blk.instructions:
                si = getattr(ins, "sync_info", None)
                if si is not None and si.on_wait and len(si.on_wait) > max_waits:
                    waits = list(si.on_wait)
                    si.on_wait = waits[-max_waits:]
                    for i, w in enumerate(waits[:-max_waits]):
                        new.append(
                            mybir.InstNoOp(
                                name=f"{ins.name}-waitfix-{i}",
                                engine=ins.engine,
                                sync_info=mybir.SyncInfo(on_wait=[w], on_update=[]),
                                bass_nofuse=True,
                            )
                        )
                        n_fix += 1
                new.append(ins)
            blk.instructions[:] = new
    return n_fix


def build_nc(n_steps):
    """Build the per-core SPMD program. n_steps = S - 1 recurrent steps.

    Two time-chains run interleaved to hide the per-step serial-dependency
    latency: chain A computes h_1..h_KA exactly; chain B computes
    h_{KA+1}..h_{n_steps}, warm-started WARM steps early from h=0 (the GRU
    contracts ~0.8/step, so the warm-start error is ~1e-7 by its first real
    step). B's warmup consumes private copies of its input blocks so its
    approximate xhat writes don't pollute chain A's inputs."""
    nc = bass.Bass()
    dt = mybir.dt
    n_chunks = (n_steps + CH - 1) // CH
    two_chain = n_steps >= 3 * WARM
    if two_chain:
        KA = (n_steps + 1 + WARM) // 2
        BN = WARM + n_steps - KA
    else:
        KA, BN = n_steps, 0

    # xm rows 0:64 = x values (block 0 pre-imputed on host), 64:128 = 1-m
    xm = nc.dram_tensor("xm", [128, n_steps, NB], dt.bfloat16, kind="ExternalInput")
    wih = nc.dram_tensor("wih", [2 * F, 3 * H], dt.bfloat16, kind="ExternalInput")
    whh = nc.dram_tensor("whh", [128, NFOLD * 3 * H], dt.bfloat16, kind="ExternalInput")
    # stacked readout: fold c -> [Wro.T fold | WoutX.T fold] = [128, 128]
    wro = nc.dram_tensor("wro", [128, NFOLD * 128], dt.bfloat16, kind="ExternalInput")
    # bst cols: 0:128 r-bias folds, 128:256 z, 256:384 gi_n, 384:512 gh_n
    # rows 0:4 = bf16-high component per fold, rows 4:8 = bf16-low remainder
    bst = nc.dram_tensor("bst", [8, 4 * 128], dt.bfloat16, kind="ExternalInput")
    ind8 = nc.dram_tensor("ind8", [8, 128], dt.bfloat16, kind="ExternalInput")
    # brop: rows 0:2 = bro hi/lo (cols 0:64), zero elsewhere; ones [2, NB]
    brop = nc.dram_tensor("brop", [2, 128], dt.bfloat16, kind="ExternalInput")
    ones = nc.dram_tensor("ones", [2, NB], dt.bfloat16, kind="ExternalInput")
    if two_chain:
        # private warmup copy of blocks KA-WARM..KA-1 (block 0 pre-imputed)
        xw = nc.dram_tensor("xw", [128, WARM, NB], dt.bfloat16,
                            kind="ExternalInput")

    op_out = nc.dram_tensor("op", [128, n_steps, NB], dt.float32, kind="ExternalOutput")

    with tile.TileContext(nc) as tc:
        with (
            tc.tile_pool(name="singles", bufs=1) as singles,
            tc.tile_pool(name="hist", bufs=1) as hist,
            tc.tile_pool(name="work", bufs=2) as work,
            tc.tile_pool(name="ps", bufs=1, space="PSUM") as psp,
            tc.tile_pool(name="outs", bufs=3) as outs,
        ):
            # --- load weights / biases (once) ---
            wih_sb = singles.tile([2 * F, 3 * H], dt.bfloat16)
            nc.sync.dma_start(out=wih_sb, in_=wih[:])
            whh_sb = singles.tile([128, NFOLD * 3 * H], dt.bfloat16)
            nc.sync.dma_start(out=whh_sb, in_=whh[:])
            wro_sb = singles.tile([128, NFOLD * 128], dt.bfloat16)
            nc.sync.dma_start(out=wro_sb, in_=wro[:])
            bst_sb = singles.tile([8, 4 * 128], dt.bfloat16)
            nc.sync.dma_start(out=bst_sb, in_=bst[:])
            ind8_sb = singles.tile([8, 128], dt.bfloat16)
            nc.sync.dma_start(out=ind8_sb, in_=ind8[:])
            brop_sb = singles.tile([2, 128], dt.bfloat16)
            nc.sync.dma_start(out=brop_sb, in_=brop[:])
            ones_sb = singles.tile([2, NB], dt.bfloat16)
            nc.sync.dma_start(out=ones_sb, in_=ones[:])

            # --- resident input, chunked so chunk 0 gates only early steps.
            # mch duplicates the 1-m rows at partitions 0:64 because
            # copy_predicated needs out/mask/data partition-aligned. ---
            xch, mch = [], []
            for c in range(n_chunks):
                c0 = c * CH
                c1 = min(n_steps, c0 + CH)
                xt = singles.tile([128, c1 - c0, NB], dt.bfloat16, name=f"xch{c}")
                nc.sync.dma_start(out=xt, in_=xm[:, c0:c1, :])
                xch.append(xt)
                mt = singles.tile([F, c1 - c0, NB], dt.bfloat16, name=f"mch{c}")
                nc.sync.dma_start(out=mt, in_=xm[F : 2 * F, c0:c1, :])
                mch.append(mt)

            def xblk(b):
                return xch[b // CH][:, b % CH, :]

            def mblk(b):
                return mch[b // CH][:, b % CH, :]

            if two_chain:
                xw_sb = singles.tile([128, WARM, NB], dt.bfloat16)
                nc.sync.dma_start(out=xw_sb, in_=xw[:])

            def whh_sl(c2, gs):
                base = c2 * 3 * H + 128 * gs
                return whh_sb[:, base : base + 128]

            def wih_sl(gs):
                return wih_sb[:, 128 * gs : 128 * (gs + 1)]

            class Chain:
                pass

            cha = Chain()
            cha.sfx, cha.n = "A", KA
            cha.blk = lambda i: i - 1
            cha.xin = lambda i: xblk(i - 1)
            cha.out_j = lambda i: i - 2
            chains = [cha]
            if two_chain:
                chb = Chain()
                chb.sfx, chb.n = "B", BN
                chb.blk = lambda i: KA - WARM + i - 1
                chb.xin = lambda i: (xw_sb[:, i - 1, :] if i <= WARM
                                     else xblk(KA - WARM + i - 1))
                chb.out_j = lambda i: (KA - WARM + i - 2
                                       if KA - WARM + i - 2 >= KA else None)
                chains.append(chb)

            for c in chains:
                # hidden state ring: [128, parity, fold*NB]; parity = t % 2
                c.h = hist.tile([128, 2, NFOLD * NB], dt.bfloat16,
                                name=f"h{c.sfx}")
                nc.vector.memset(c.h[:, 0, :], 0.0)
                c.hfold = (lambda cc: lambda pv, c2:
                           cc.h[:, pv, c2 * NB : (c2 + 1) * NB])(c)

            def emit_readout(c, i, pv, tail=False):
                """Readout of h_{i-1} (or h_n for tail): psum <- bro +
                [Wro|WoutX]^T h (bro via K=2 seed matmul); predicated xhat
                overwrite into x block straight from PSUM. The SBUF copy +
                DMA for exact outputs is deferred to phase2 (off the
                critical path). Returns ps_ro."""
                ps_ro = psp.tile([128, NB], dt.float32, tag="ro", bufs=2,
                                 padded_shape=[128, 512], name=f"ro{c.sfx}{i}")
                nc.tensor.matmul(ps_ro, brop_sb, ones_sb, start=True, stop=False,
                                 skip_group_check=True)
                for c2 in range(NFOLD):
                    nc.tensor.matmul(ps_ro, wro_sb[:, c2 * 128 : (c2 + 1) * 128],
                                     c.hfold(pv, c2), start=False,
                                     stop=(c2 == NFOLD - 1),
                                     skip_group_check=True)
                if not tail:
                    nc.vector.copy_predicated(
                        c.xin(i)[0:F, :],
                        mblk(c.blk(i)).bitcast(mybir.dt.uint16),
                        ps_ro[0:F, :],
                    )
                return ps_ro

            def emit_out(c, i, ps_ro, tail=False):
                out_j = (c.n - 1 + (KA - WARM if c.sfx == "B" else 0)) if tail \
                    else c.out_j(i)
                if out_j is not None:
                    out_t = outs.tile([128, NB], dt.float32, tag="out_t",
                                      name=f"out{c.sfx}{i}")
                    nc.scalar.activation(out=out_t, in_=ps_ro, func=AF.Copy)
                    nc.sync.dma_start(out=op_out[:, out_j, :], in_=out_t)

            def phase1(c, i):
                """Readout + predicated-impute + the full matmul stream."""
                pv = (i - 1) % 2
                ps_ro = emit_readout(c, i, pv) if i >= 2 else None
                x_in = c.xin(i)

                # per-chain banks so start=True seeds never WAR on the other
                # chain's readers (head-of-line stall in the tensor FIFO);
                # Ngh+Ngi share one bank (regions 0:128 / 128:256).
                rbk = psp.tile([128, NFOLD * NB], dt.float32, tag=f"rbank{c.sfx}",
                               padded_shape=[128, 512], name=f"rb{c.sfx}{i}")
                zbk = psp.tile([128, NFOLD * NB], dt.float32, tag=f"zbank{c.sfx}",
                               padded_shape=[128, 512], name=f"zb{c.sfx}{i}")
                nnb = psp.tile([128, 2 * NFOLD * NB], dt.float32, tag=f"nnbank{c.sfx}",
                               padded_shape=[128, 512], name=f"nn{c.sfx}{i}")
                ngh = nnb[:, 0 : NFOLD * NB]
                ngi = nnb[:, NFOLD * NB : 2 * NFOLD * NB]

                def seed(reg, bias_col, start):
                    nc.tensor.matmul(
                        reg, bst_sb[:, bias_col * 128 : (bias_col + 1) * 128],
                        ind8_sb[:, 0 : NFOLD * NB],
                        start=start, stop=False, skip_group_check=True,
                    )

                def bank_mms(bk, bias_col, gs0, with_gi, last_stop):
                    seed(bk[:, 0 : NFOLD * NB], bias_col, start=True)
                    for s in range(NFOLD):
                        reg = bk[:, s * NB : (s + 1) * NB]
                        for c2 in range(NFOLD):
                            nc.tensor.matmul(
                                reg, whh_sl(c2, gs0 + s), c.hfold(pv, c2),
                                start=False,
                                stop=(last_stop and not with_gi
                                      and s == NFOLD - 1 and c2 == NFOLD - 1),
                                skip_group_check=True,
                            )
                    if with_gi:
                        for s in range(NFOLD):
                            reg = bk[:, s * NB : (s + 1) * NB]
                            nc.tensor.matmul(
                                reg, wih_sl(gs0 + s), x_in,
                                start=False, stop=(last_stop and s == NFOLD - 1),
                                skip_group_check=True,
                            )

                # tensor stream: R -> N (gh region + gi region) -> Z(last)
                bank_mms(rbk, 0, 0, with_gi=True, last_stop=True)
                bank_mms(nnb, 3, 8, with_gi=False, last_stop=False)
                seed(ngi, 2, start=False)
                for s in range(NFOLD):
                    nc.tensor.matmul(
                        ngi[:, s * NB : (s + 1) * NB], wih_sl(8 + s), x_in,
                        start=False, stop=(s == NFOLD - 1),
                        skip_group_check=True,
                    )
                bank_mms(zbk, 1, 4, with_gi=True, last_stop=True)
                c.cur = (i, ps_ro, rbk, zbk, ngh, ngi)

            def phase2(c):
                """Gate nonlinearities + state update + deferred output."""
                i, ps_ro, rbk, zbk, ngh, ngi = c.cur
                pv, cur = (i - 1) % 2, i % 2
                r_t = work.tile([128, NFOLD * NB], dt.bfloat16,
                                tag=f"r_t{c.sfx}", name=f"r{c.sfx}{i}")
                nc.scalar.activation(out=r_t, in_=rbk, func=AF.Sigmoid)
                nin1 = work.tile([128, NFOLD * NB], dt.float32,
                                 tag=f"nin1{c.sfx}", name=f"n1{c.sfx}{i}")
                nc.vector.tensor_tensor(nin1, ngh, r_t, ALU.mult)
                nin2 = work.tile([128, NFOLD * NB], dt.float32,
                                 tag=f"nin2{c.sfx}", name=f"n2{c.sfx}{i}")
                nc.vector.tensor_tensor(nin2, nin1, ngi, ALU.add)

                z_t = work.tile([128, NFOLD * NB], dt.bfloat16,
                                tag=f"z_t{c.sfx}", name=f"z{c.sfx}{i}")
                nc.scalar.activation(out=z_t, in_=zbk, func=AF.Sigmoid)
                n_t = work.tile([128, NFOLD * NB], dt.bfloat16,
                                tag=f"n_t{c.sfx}", name=f"n{c.sfx}{i}")
                nc.scalar.activation(out=n_t, in_=nin2, func=AF.Tanh)

                omz = work.tile([128, NFOLD * NB], dt.bfloat16,
                                tag=f"omz{c.sfx}", name=f"om{c.sfx}{i}")
                nc.gpsimd.tensor_scalar(omz, z_t, -1.0, 1.0, ALU.mult, ALU.add)
                zh = work.tile([128, NFOLD * NB], dt.bfloat16,
                               tag=f"zh{c.sfx}", name=f"zh{c.sfx}{i}")
                nc.gpsimd.tensor_tensor(zh, z_t, c.h[:, pv, :], ALU.mult)
                t3 = work.tile([128, NFOLD * NB], dt.bfloat16,
                               tag=f"t3{c.sfx}", name=f"t3{c.sfx}{i}")
                nc.vector.tensor_tensor(t3, n_t, omz, ALU.mult)
                nc.vector.tensor_tensor(c.h[:, cur, :], t3, zh, ALU.add)
                if ps_ro is not None:
                    emit_out(c, i, ps_ro)

            # software-pipelined emission: each chain's matmul stream is
            # emitted between the other chain's phase1 and phase2, so the
            # per-engine FIFO order matches the intended interleaved
            # schedule (A-stream || B-chain, then B-stream || A-chain).
            cb = chains[1] if two_chain else None
            for it in range(1, cha.n + 1):
                phase1(cha, it)
                if cb is not None and it >= 2 and it - 1 <= cb.n:
                    phase2(cb)       # covers all of B: BN <= KA-1
                if cb is not None and it <= cb.n:
                    phase1(cb, it)
                phase2(cha)
            for c in chains:
                ps_ro = emit_readout(c, c.n + 1, c.n % 2, tail=True)
                emit_out(c, c.n + 1, ps_ro, tail=True)

    _legalize_multiwait(nc)
    return nc


_NC_CACHE = {}


def _get_nc(n_steps):
    if n_steps not in _NC_CACHE:
        _NC_CACHE[n_steps] = build_nc(n_steps)
    return _NC_CACHE[n_steps]


def _prep_core_inputs(x2d, m2d, Wih, Whh, bih, bhh, Wro, bro, Wout_half, n_steps):
    """Per-core input map. x2d/m2d: [NB, S_loc, F] float32/bool already
    direction-ordered (time-reversed for backward cores)."""
    Wih = np.asarray(Wih, np.float32)
    bih = np.asarray(bih, np.float32)
    bhh = np.asarray(bhh, np.float32)
    bro_f = np.asarray(bro, np.float32)

    xt = np.ascontiguousarray(x2d[:, :n_steps].transpose(2, 1, 0)).astype(np.float32)
    mt = m2d[:, :n_steps].transpose(2, 1, 0)          # [F, t, NB] bool
    # block 0 x-rows pre-imputed with bro (xhat_0); mask rows hold 1-m
    xt[:, 0, :] = np.where(mt[:, 0, :], xt[:, 0, :], bro_f[:, None])
    xm = np.concatenate([xt, 1.0 - mt.astype(np.float32)], axis=0).astype(BF16)

    extra = {}
    if n_steps >= 3 * WARM:
        ka = (n_steps + 1 + WARM) // 2
        xw_f = xt[:, ka - WARM : ka].copy()           # [F, WARM, NB]
        mw = mt[:, ka - WARM : ka]
        xw_f[:, 0] = np.where(mw[:, 0], xw_f[:, 0], bro_f[:, None])
        extra["xw"] = np.concatenate(
            [xw_f, 1.0 - mw.astype(np.float32)], axis=0).astype(BF16)

    wih_t = Wih.T.copy()                               # [2F, 3H]
    wih_t[F:] = -wih_t[F:]                             # mask half negated
    wih_t = np.ascontiguousarray(wih_t).astype(BF16)
    whh_t = np.ascontiguousarray(
        np.asarray(Whh, np.float32).T.reshape(NFOLD, 128, 3 * H)
        .transpose(1, 0, 2).reshape(128, NFOLD * 3 * H)
    ).astype(BF16)
    wro_f = np.asarray(Wro, np.float32).T.reshape(NFOLD, 128, F)
    wout_f = np.asarray(Wout_half, np.float32).T.reshape(NFOLD, 128, F)
    wro_t = np.ascontiguousarray(
        np.concatenate([wro_f, wout_f], axis=2)
        .transpose(1, 0, 2).reshape(128, NFOLD * 128)
    ).astype(BF16)

    # biases with the mask-rowsum adjustment (m = 1 - inv_m)
    radj = Wih[:, F:].sum(axis=1)                      # [3H]
    bsum = bih + bhh + radj
    b_r, b_z = bsum[0:H], bsum[H : 2 * H]
    b_in = bih[2 * H :] + radj[2 * H :]
    b_hn = bhh[2 * H :]
    bst_f = np.empty((4, 4 * 128), np.float32)
    for k in range(4):
        bst_f[k, 0:128] = b_r[128 * k : 128 * (k + 1)]
        bst_f[k, 128:256] = b_z[128 * k : 128 * (k + 1)]
        bst_f[k, 256:384] = b_in[128 * k : 128 * (k + 1)]
        bst_f[k, 384:512] = b_hn[128 * k : 128 * (k + 1)]
    # hi/lo bf16 split: rows 0:4 = bf16(b), rows 4:8 = bf16(b - hi)
    bst = np.empty((8, 4 * 128), BF16)
    bst[0:4] = bst_f.astype(BF16)
    bst[4:8] = (bst_f - bst[0:4].astype(np.float32)).astype(BF16)
    ind8 = np.zeros((8, 128), np.float32)
    for k in range(4):
        ind8[k, 32 * k : 32 * (k + 1)] = 1.0
        ind8[4 + k, 32 * k : 32 * (k + 1)] = 1.0
    brop_f = np.zeros((2, 128), np.float32)
    brop_f[0, 0:F] = bro_f
    brop = np.empty((2, 128), BF16)
    brop[0] = brop_f[0].astype(BF16)
    brop[1] = (brop_f[0] - brop[0].astype(np.float32)).astype(BF16)

    return {
        "xm": xm, "wih": wih_t, "whh": whh_t, "wro": wro_t,
        "bst": bst, "ind8": ind8.astype(BF16), "brop": brop,
        "ones": np.ones((2, NB), BF16), **extra,
    }


def run_device(inputs, s_len=S, trace=False):
    """Run the 8-core SPMD kernel. Returns BassKernelResults."""
    n_steps = s_len - 1
    nc = _get_nc(n_steps)

    x2d = np.asarray(inputs["x"], np.float32).reshape(B, S, F)[:, :s_len]
    m2d = np.asarray(inputs["mask"]).reshape(B, S, F)[:, :s_len]

    in_maps = []
    for core in range(8):
        g = core % 4
        bsl = slice(NB * g, NB * (g + 1))
        if core < 4:
            im = _prep_core_inputs(
                x2d[bsl], m2d[bsl], inputs["Wih_f"], inputs["Whh_f"],
                inputs["bih_f"], inputs["bhh_f"], inputs["Wro_f"], inputs["bro_f"],
                np.asarray(inputs["Wout"])[:, :H], n_steps,
            )
        else:
            im = _prep_core_inputs(
                x2d[bsl, ::-1], m2d[bsl, ::-1], inputs["Wih_b"], inputs["Whh_b"],
                inputs["bih_b"], inputs["bhh_b"], inputs["Wro_b"], inputs["bro_b"],
                np.asarray(inputs["Wout"])[:, H:], n_steps,
            )
        in_maps.append(im)

    return run_bass_kernel_spmd(nc, in_maps, core_ids=list(range(8)), trace=trace)


def assemble(inputs, res, s_len=S):
    """Host-side gather: combine per-core outputs into full reference outputs."""
    n_steps = s_len - 1
    bro_f = np.asarray(inputs["bro_f"], np.float32)
    bro_b = np.asarray(inputs["bro_b"], np.float32)
    bout = np.asarray(inputs["bout"], np.float32)

    xh_f = np.empty((B, s_len, F), np.float32)
    xh_b = np.empty((B, s_len, F), np.float32)
    x_hat = np.empty((B, s_len, F), np.float32)

    for g in range(4):
        bsl = slice(NB * g, NB * (g + 1))
        rf, rb = res.results[g], res.results[g + 4]
        # device output "op" is [128, n_steps, NB]: rows 0:64 xhat, 64:128 pp
        xh_f[bsl, 1:] = rf["op"][:F].transpose(2, 1, 0)
        xh_f[bsl, 0] = bro_f
        xh_b[bsl, :n_steps] = rb["op"][:F].transpose(2, 1, 0)[:, ::-1]
        xh_b[bsl, n_steps] = bro_b
        pf = rf["op"][F:].transpose(2, 1, 0)
        pb = rb["op"][F:].transpose(2, 1, 0)[:, ::-1]
        x_hat[bsl, 1:] = pf
        x_hat[bsl, 0] = 0.0
        x_hat[bsl, :n_steps] += pb
        x_hat[bsl] += bout

    return (
        x_hat.reshape(B, s_len, N, C),
        xh_f.reshape(B, s_len, N, C),
        xh_b.reshape(B, s_len, N, C),
    )


def kernel(**inputs):
    res = run_device(inputs, s_len=S)
    return assemble(inputs, res, s_len=S)
